# revision 63
# baseline (speedup 1.0000x reference)
"""Multi-head attention (CTRL-style causal) on 8 TRN2 NeuronCores.

Sharding: core = b*4 + g  (b in {0,1} batch, g in {0..3} head-group of 4 heads).
Each core computes projections for its 4 heads on its batch, causal attention,
and a partial dense output (row-sharded dense). Host sums the 4 partials per
batch and adds dense_b.

Self-contained: hardcodes all shapes; imports only installed packages.
"""

import numpy as np
import ml_dtypes

B, S, D, H = 2, 2048, 1024, 16
HG = 4            # heads per core
DH = 64           # head depth
DG = HG * DH      # 256 projection width per core
NCORES = 8
QC = 4            # 512-wide query chunks
KT = S // 128     # 16 key tiles
MT = 2            # 128-row tiles of DG

_CACHE = {}


def _build_nc():
    import concourse.mybir as mybir
    import concourse.tile as tile
    from concourse import bacc

    FP32 = mybir.dt.float32
    CDT = mybir.dt.bfloat16
    Exp = mybir.ActivationFunctionType.Exp

    nc = bacc.Bacc("TRN2", target_bir_lowering=False, debug=False,
                   num_devices=NCORES)

    # x and w are host-reordered to partition-major k-tile layout:
    # x[p, k*S + s] = x_T[k*128 + p, s] — one DMA with 32KB/partition
    # contiguous descriptors instead of 8 DMAs at 4KB/partition.
    xqT = nc.dram_tensor("xqT", [128, 8 * S], CDT, kind="ExternalInput")
    xkT = nc.dram_tensor("xkT", [128, 8 * S], CDT, kind="ExternalInput")
    xvT = nc.dram_tensor("xvT", [128, 8 * S], CDT, kind="ExternalInput")
    wq = nc.dram_tensor("wq", [128, 8 * DG], CDT, kind="ExternalInput")
    wk = nc.dram_tensor("wk", [128, 8 * DG], CDT, kind="ExternalInput")
    wv = nc.dram_tensor("wv", [128, 8 * DG], CDT, kind="ExternalInput")
    bq = nc.dram_tensor("bq", [128, MT], FP32, kind="ExternalInput")
    bk = nc.dram_tensor("bk", [128, MT], FP32, kind="ExternalInput")
    wd = nc.dram_tensor("wd", [DG, D], CDT, kind="ExternalInput")
    out = nc.dram_tensor("out", [S, D], CDT, kind="ExternalOutput")
    wout = nc.dram_tensor("wout", [1, 64], FP32, kind="ExternalOutput")

    with tile.TileContext(nc) as tc:
        with (
            tc.tile_pool(name="const", bufs=1) as const,
            tc.tile_pool(name="wpool", bufs=1) as wpool,
            tc.tile_pool(name="xpool", bufs=3) as xpool,
            tc.tile_pool(name="qkpool", bufs=1) as qkpool,
            tc.tile_pool(name="vpool", bufs=1) as vpool,
            tc.tile_pool(name="spool", bufs=8) as spool,
            tc.tile_pool(name="small", bufs=6) as small,
            tc.tile_pool(name="opool", bufs=6) as opool,
            tc.tile_pool(name="sppool", bufs=2, space="PSUM") as sppool,
            tc.tile_pool(name="pcpool", bufs=2, space="PSUM") as pcpool,
            tc.tile_pool(name="pdpool", bufs=2, space="PSUM") as pdpool,
        ):
            # ---- PE warmup: ~10us of dummy matmuls so HAM reaches
            # K=8/8 before the real work starts (overlaps input DMA).
            wsrc = const.tile([128, 512], CDT, name="wsrc")
            nc.vector.memset(wsrc[:], 0.001)
            wps = pdpool.tile([128, 512], FP32, tag="pd", name="wps")
            NWARM = 52
            for i in range(NWARM):
                nc.tensor.matmul(wps[:], wsrc[:, 0:128], wsrc[:],
                                 start=(i == 0), stop=(i == NWARM - 1))
            wsb = const.tile([1, 64], FP32, name="wsb")
            nc.vector.tensor_copy(wsb[:], wps[0:1, 0:64])

            # ---- constants first (tiny; must not queue behind bulk) --
            bqt = const.tile([128, MT], FP32, name="bqt")
            nc.sync.dma_start(bqt[:], bq[:])
            bkt = const.tile([128, MT], FP32, name="bkt")
            nc.sync.dma_start(bkt[:], bk[:])

            # ---- weights + activations (one DMA per tensor; k-tiles
            # are views). Use-order, and the three big x transfers are
            # chained so each gets full DMA bandwidth in turn instead
            # of round-robin sharing (xq must land first).
            def load_w(name, dram, cols):
                t = wpool.tile([128, 8 * cols], CDT, name=name)
                nc.sync.dma_start(t[:], dram[:])
                return [t[:, k * cols:(k + 1) * cols] for k in range(8)]

            from concourse.tile import add_dep_helper

            wqt = load_w("wqt", wq, DG)
            xqa = xpool.tile([128, 8 * S], CDT, tag="xt", name="xqa")
            dq = nc.sync.dma_start(xqa[:], xqT[:])
            wkt = load_w("wkt", wk, DG)
            xka = xpool.tile([128, 8 * S], CDT, tag="xt", name="xka")
            dk = nc.sync.dma_start(xka[:], xkT[:])
            wvt = load_w("wvt", wv, DG)
            xva = xpool.tile([128, 8 * S], CDT, tag="xt", name="xva")
            dv = nc.sync.dma_start(xva[:], xvT[:])
            add_dep_helper(dk.ins, dq.ins, sync=True, reason="stagger x dma")
            add_dep_helper(dv.ins, dk.ins, sync=True, reason="stagger x dma")
            wdt = []
            for k in range(MT):
                t = wpool.tile([128, D], CDT, name=f"wdt{k}")
                nc.sync.dma_start(t[:], wd[k * 128:(k + 1) * 128, :])
                wdt.append(t)

            # keep 1.0 where q_local >= k_local else 0.0 (diag 128-block)
            trimask = const.tile([128, 128], CDT, name="trimask")
            nc.vector.memset(trimask[:], 1.0)
            nc.gpsimd.affine_select(
                out=trimask[:], in_=trimask[:],
                compare_op=mybir.AluOpType.is_ge,
                fill=0.0, base=0,
                pattern=[[1, 128]], channel_multiplier=-1,
            )

            # ---- projections -----------------------------------------
            qT = [qkpool.tile([128, S], CDT, name=f"qT{m}") for m in range(MT)]
            kTt = [qkpool.tile([128, S], CDT, name=f"kT{m}") for m in range(MT)]
            ctxT = [qkpool.tile([128, S], CDT, name=f"ctxT{m}") for m in range(MT)]

            xqt = [xqa[:, k * S:(k + 1) * S] for k in range(8)]
            xkt = [xka[:, k * S:(k + 1) * S] for k in range(8)]
            xvt = [xva[:, k * S:(k + 1) * S] for k in range(8)]
            vaug = [vpool.tile([128, HG * 65], CDT, name=f"vaug{st}")
                    for st in range(KT)]

            def proj_qk_group(xt, wt, bt, dst, m, c):
                pt = pdpool.tile([128, 512], FP32, tag="pd", name="pt")
                for k in range(8):
                    nc.tensor.matmul(
                        pt[:],
                        wt[k][:, m * 128:(m + 1) * 128],
                        xt[k][:, c * 512:(c + 1) * 512],
                        start=(k == 0), stop=(k == 7),
                    )
                # eviction with per-partition bias on DVE
                nc.vector.tensor_scalar_add(
                    dst[m][:, c * 512:(c + 1) * 512], pt[:], bt[:, m:m + 1])

            def proj_v_group(st):
                # V in natural [seq, head-dim] layout, a ones column
                # appended per head for the softmax denominator.
                pv = pdpool.tile([128, 512], FP32, tag="pd", name="pv")
                for k in range(8):
                    nc.tensor.matmul(
                        pv[:, :DG],
                        xvt[k][:, st * 128:(st + 1) * 128],
                        wvt[k][:],
                        start=(k == 0), stop=(k == 7),
                    )
                va = vaug[st].rearrange("p (h c) -> p h c", h=HG)
                nc.vector.tensor_copy(
                    va[:, :, 0:DH],
                    pv[:, :DG].rearrange("p (h c) -> p h c", h=HG),
                )
                nc.vector.memset(va[:, :, DH:DH + 1], 1.0)

            # Q fully; K and V only what attention qc=0 needs. The rest
            # is emitted inside the qc loop so it fills ACT-bound stalls.
            for c in range(QC):
                for m in range(MT):
                    proj_qk_group(xqt, wqt, bqt, qT, m, c)
            for c in range(QC):
                for m in range(MT):
                    proj_qk_group(xkt, wkt, bkt, kTt, m, c)

            # dep-free filler: bridges PE idle between the end of
            # K-projection and the xv DMA arrival (uses an attention
            # PSUM slot, idle during projections).
            wps2 = sppool.tile([128, 1024], FP32, tag="sp", name="wps2")
            NFILL = 24
            for i in range(NFILL):
                nc.tensor.matmul(wps2[:, 0:512], wsrc[:, 0:128], wsrc[:],
                                 start=(i == 0), stop=(i == NFILL - 1))
            nc.vector.tensor_copy(wsb[:], wps2[0:1, 0:64])

            for st in range(4):
                proj_v_group(st)

            def dense_block(qt):
                for n in range(2):
                    dps = pdpool.tile([128, 512], FP32, tag="pd", name="dps")
                    for km in range(MT):
                        nc.tensor.matmul(
                            dps[:],
                            ctxT[km][:, qt * 128:(qt + 1) * 128],
                            wdt[km][:, n * 512:(n + 1) * 512],
                            start=(km == 0), stop=(km == MT - 1),
                        )
                    ot = opool.tile([128, 512], CDT, tag="ot", name="ot")
                    if n == 0:
                        nc.scalar.copy(ot[:], dps[:])
                    else:
                        nc.vector.tensor_copy(ot[:], dps[:])
                    nc.sync.dma_start(
                        out[qt * 128:(qt + 1) * 128, n * 512:(n + 1) * 512],
                        ot[:],
                    )

            # ---- attention + dense, interleaved per q-chunk ----------
            for qc in range(QC):
                nkt = 4 * qc + 4       # causal k tiles for this q chunk
                lastkt = nkt - 1
                for h in range(HG):
                    m, po = h // 2, 64 * (h % 2)
                    cps = pcpool.tile([65, 512], FP32, tag="cps", name="cps")

                    def emit_s(kp):
                        sp = sppool.tile([128, 1024], FP32, tag="sp", name="sp")
                        for j in range(2):
                            kt = 2 * kp + j
                            r = kt - 4 * qc
                            off = 128 * r if r > 0 else 0
                            nc.tensor.matmul(
                                sp[:, j * 512 + off:(j + 1) * 512],
                                kTt[m][po:po + DH, kt * 128:(kt + 1) * 128],
                                qT[m][po:po + DH,
                                      qc * 512 + off:(qc + 1) * 512],
                            )
                        return sp

                    def emit_ec(kp, sp):
                        es = spool.tile([128, 1024], CDT, tag="es", name="es")
                        offs = [max(0, 128 * (2 * kp + j - 4 * qc))
                                for j in range(2)]
                        if offs == [0, 0]:
                            nc.scalar.activation(es[:], sp[:], Exp, scale=0.125)
                        else:
                            for j in range(2):
                                sl = slice(j * 512 + offs[j], (j + 1) * 512)
                                nc.scalar.activation(es[:, sl], sp[:, sl],
                                                     Exp, scale=0.125)
                        for j in range(2):
                            kt = 2 * kp + j
                            r = kt - 4 * qc
                            off = 128 * r if r > 0 else 0
                            if r >= 0:
                                dj = es[:, j * 512 + off:j * 512 + off + 128]
                                nc.vector.tensor_mul(dj, dj, trimask[:])
                            nc.tensor.matmul(
                                cps[:, off:512],
                                vaug[kt][:, 65 * h:65 * h + 65],
                                es[:, j * 512 + off:(j + 1) * 512],
                                start=(kt == 0), stop=(kt == lastkt),
                            )

                    # software pipeline: S-pairs run one stage ahead of
                    # the exp/C chain so PE never waits on ACT latency.
                    prev = emit_s(0)
                    for kp in range(1, nkt // 2):
                        sp = emit_s(kp)
                        emit_ec(kp - 1, prev)
                        prev = sp
                    emit_ec(nkt // 2 - 1, prev)

                    den = small.tile([1, 512], FP32, tag="den", name="den")
                    if qc < 2:
                        # early q-chunks: DVE is the loaded engine, ACT
                        # has slack — put the denominator copy there.
                        nc.scalar.copy(den[:], cps[DH:DH + 1, :])
                    else:
                        nc.vector.tensor_copy(den[:], cps[DH:DH + 1, :])
                    recip = small.tile([1, 512], FP32, tag="recip", name="recip")
                    nc.vector.reciprocal_approx_fast(recip[:], den[:])
                    recipb = small.tile([DH, 512], FP32, tag="recipb", name="recipb")
                    nc.gpsimd.partition_broadcast(recipb[:], recip[:])
                    cslice = ctxT[m][po:po + DH, qc * 512:(qc + 1) * 512]
                    nc.vector.tensor_mul(cslice, cps[0:DH, :], recipb[:])
                    # fine-grained interleave: one V-prefetch group and
                    # one deferred dense block per head keeps ACT fed
                    # with exp work instead of idling through long
                    # PE-only bursts between q-chunks.
                    if qc < QC - 1:
                        proj_v_group(4 * qc + 4 + h)
                    if qc > 0:
                        dense_block(4 * (qc - 1) + h)

            # dep-free filler: keeps the PE warm while the last head's
            # normalize chain drains before the final dense blocks.
            wps3 = sppool.tile([128, 1024], FP32, tag="sp", name="wps3")
            for i in range(20):
                nc.tensor.matmul(wps3[:, 0:512], wsrc[:, 0:128], wsrc[:],
                                 start=(i == 0), stop=(i == 19))
            nc.vector.tensor_copy(wsb[:], wps3[0:1, 0:64])
            for qt in range(12, 16):
                dense_block(qt)

            # warmup result DMA last, on the gpsimd queue, so it cannot
            # stall input DMAs on the sync queue.
            nc.gpsimd.dma_start(wout[:], wsb[:])

    nc.compile()
    return nc


def get_nc():
    if "nc" not in _CACHE:
        _CACHE["nc"] = _build_nc()
    return _CACHE["nc"]


def make_in_maps(q, k, v, Wq_w, Wq_b, Wk_w, Wk_b, Wv_w, Wv_b, dense_w):
    bf16 = ml_dtypes.bfloat16
    f32 = np.float32

    def cast_t(a):
        # [S, D] -> transposed [D, S] -> partition-major k-tile layout
        # [128, 8*S] with row p holding k-tiles 0..7's partition p.
        t = np.asarray(a, f32).T.reshape(8, 128, S).transpose(1, 0, 2)
        return np.ascontiguousarray(t.reshape(128, 8 * S)).astype(bf16)

    def cast_w(wmat, gsl):
        # W[gsl].T [D, DG] -> [128, 8*DG] partition-major k-tiles
        t = np.asarray(wmat, f32)[gsl, :].T.reshape(8, 128, DG)
        return np.ascontiguousarray(
            t.transpose(1, 0, 2).reshape(128, 8 * DG)).astype(bf16)

    xq = [cast_t(q[b]) for b in range(B)]
    xk = [cast_t(k[b]) for b in range(B)]
    xv = [cast_t(v[b]) for b in range(B)]

    in_maps = []
    for core in range(NCORES):
        b, g = core // 4, core % 4
        gsl = slice(g * DG, (g + 1) * DG)
        m = {
            "xqT": xq[b], "xkT": xk[b], "xvT": xv[b],
            "wq": cast_w(Wq_w, gsl),
            "wk": cast_w(Wk_w, gsl),
            "wv": cast_w(Wv_w, gsl),
            "bq": np.ascontiguousarray(np.asarray(Wq_b, f32)[gsl].reshape(MT, 128).T),
            "bk": np.ascontiguousarray(np.asarray(Wk_b, f32)[gsl].reshape(MT, 128).T),
            "wd": np.ascontiguousarray(np.asarray(dense_w, f32)[:, gsl].T).astype(bf16),
        }
        in_maps.append(m)
    return in_maps


def gather(results, dense_b, Wv_b, dense_w):
    # V-bias is linear through the dense layer: ctx bias contributes the
    # constant row Wv_b @ dense_w.T, added here instead of on device.
    const_row = (np.asarray(Wv_b, np.float32) @
                 np.asarray(dense_w, np.float32).T + np.asarray(dense_b, np.float32))
    out = np.empty((B, S, D), np.float32)
    for b in range(B):
        acc = results[b * 4]["out"].astype(np.float32).copy()
        for g in range(1, 4):
            acc += results[b * 4 + g]["out"]
        out[b] = acc + const_row
    return out


def kernel(q, k, v, mask, Wq_w, Wq_b, Wk_w, Wk_b, Wv_w, Wv_b, dense_w, dense_b,
           trace=False):
    from concourse.bass_utils import run_bass_kernel_spmd

    nc = get_nc()
    in_maps = make_in_maps(q, k, v, Wq_w, Wq_b, Wk_w, Wk_b, Wv_w, Wv_b, dense_w)
    res = run_bass_kernel_spmd(nc, in_maps, list(range(NCORES)), trace=trace)
    out = gather(res.results, dense_b, Wv_b, dense_w)
    if trace:
        return out, res
    return out


# revision 64
# speedup vs baseline: 1.0330x; 1.0330x over previous
"""Multi-head attention (CTRL-style causal) on 8 TRN2 NeuronCores.

Sharding: core = b*4 + g  (b in {0,1} batch, g in {0..3} head-group of 4 heads).
Each core computes projections for its 4 heads on its batch, causal attention,
and a partial dense output (row-sharded dense). Host sums the 4 partials per
batch and adds dense_b.

Self-contained: hardcodes all shapes; imports only installed packages.
"""

import numpy as np
import ml_dtypes

B, S, D, H = 2, 2048, 1024, 16
HG = 4            # heads per core
DH = 64           # head depth
DG = HG * DH      # 256 projection width per core
NCORES = 8
QC = 4            # 512-wide query chunks
KT = S // 128     # 16 key tiles
MT = 2            # 128-row tiles of DG

_CACHE = {}


def _build_nc():
    import concourse.mybir as mybir
    import concourse.tile as tile
    from concourse import bacc

    FP32 = mybir.dt.float32
    CDT = mybir.dt.bfloat16
    Exp = mybir.ActivationFunctionType.Exp

    nc = bacc.Bacc("TRN2", target_bir_lowering=False, debug=False,
                   num_devices=NCORES)

    # x and w are host-reordered to partition-major k-tile layout:
    # x[p, k*S + s] = x_T[k*128 + p, s] — one DMA with 32KB/partition
    # contiguous descriptors instead of 8 DMAs at 4KB/partition.
    xqT = nc.dram_tensor("xqT", [128, 8 * S], CDT, kind="ExternalInput")
    xkT = nc.dram_tensor("xkT", [128, 8 * S], CDT, kind="ExternalInput")
    xvT = nc.dram_tensor("xvT", [128, 8 * S], CDT, kind="ExternalInput")
    wq = nc.dram_tensor("wq", [128, 8 * DG], CDT, kind="ExternalInput")
    wk = nc.dram_tensor("wk", [128, 8 * DG], CDT, kind="ExternalInput")
    wv = nc.dram_tensor("wv", [128, 8 * DG], CDT, kind="ExternalInput")
    bq = nc.dram_tensor("bq", [128, MT], FP32, kind="ExternalInput")
    bk = nc.dram_tensor("bk", [128, MT], FP32, kind="ExternalInput")
    wd = nc.dram_tensor("wd", [DG, D], CDT, kind="ExternalInput")
    out = nc.dram_tensor("out", [S, D], CDT, kind="ExternalOutput")
    wout = nc.dram_tensor("wout", [1, 64], FP32, kind="ExternalOutput")

    with tile.TileContext(nc) as tc:
        with (
            tc.tile_pool(name="const", bufs=1) as const,
            tc.tile_pool(name="wpool", bufs=1) as wpool,
            tc.tile_pool(name="xpool", bufs=3) as xpool,
            tc.tile_pool(name="qkpool", bufs=1) as qkpool,
            tc.tile_pool(name="vpool", bufs=1) as vpool,
            tc.tile_pool(name="spool", bufs=8) as spool,
            tc.tile_pool(name="small", bufs=6) as small,
            tc.tile_pool(name="opool", bufs=6) as opool,
            tc.tile_pool(name="sppool", bufs=2, space="PSUM") as sppool,
            tc.tile_pool(name="pcpool", bufs=2, space="PSUM") as pcpool,
            tc.tile_pool(name="pdpool", bufs=2, space="PSUM") as pdpool,
        ):
            # ---- PE warmup: ~10us of dummy matmuls so HAM reaches
            # K=8/8 before the real work starts (overlaps input DMA).
            wsrc = const.tile([128, 512], CDT, name="wsrc")
            nc.vector.memset(wsrc[:], 0.001)
            wps = pdpool.tile([128, 512], FP32, tag="pd", name="wps")
            NWARM = 52
            for i in range(NWARM):
                nc.tensor.matmul(wps[:], wsrc[:, 0:128], wsrc[:],
                                 start=(i == 0), stop=(i == NWARM - 1))
            wsb = const.tile([1, 64], FP32, name="wsb")
            nc.vector.tensor_copy(wsb[:], wps[0:1, 0:64])

            # ---- constants first (tiny; must not queue behind bulk) --
            bqt = const.tile([128, MT], FP32, name="bqt")
            nc.sync.dma_start(bqt[:], bq[:])
            bkt = const.tile([128, MT], FP32, name="bkt")
            nc.sync.dma_start(bkt[:], bk[:])

            # ---- weights + activations (one DMA per tensor; k-tiles
            # are views). Use-order, and the three big x transfers are
            # chained so each gets full DMA bandwidth in turn instead
            # of round-robin sharing (xq must land first).
            def load_w(name, dram, cols):
                t = wpool.tile([128, 8 * cols], CDT, name=name)
                nc.sync.dma_start(t[:], dram[:])
                return [t[:, k * cols:(k + 1) * cols] for k in range(8)]

            from concourse.tile import add_dep_helper

            wqt = load_w("wqt", wq, DG)
            xqa = xpool.tile([128, 8 * S], CDT, tag="xt", name="xqa")
            dq = nc.sync.dma_start(xqa[:], xqT[:])
            wkt = load_w("wkt", wk, DG)
            xka = xpool.tile([128, 8 * S], CDT, tag="xt", name="xka")
            dk = nc.sync.dma_start(xka[:], xkT[:])
            wvt = load_w("wvt", wv, DG)
            xva = xpool.tile([128, 8 * S], CDT, tag="xt", name="xva")
            dv = nc.sync.dma_start(xva[:], xvT[:])
            add_dep_helper(dk.ins, dq.ins, sync=True, reason="stagger x dma")
            add_dep_helper(dv.ins, dk.ins, sync=True, reason="stagger x dma")
            wdt = []
            for k in range(MT):
                t = wpool.tile([128, D], CDT, name=f"wdt{k}")
                nc.sync.dma_start(t[:], wd[k * 128:(k + 1) * 128, :])
                wdt.append(t)

            # keep 1.0 where q_local >= k_local else 0.0 (diag 128-block)
            trimask = const.tile([128, 128], CDT, name="trimask")
            nc.vector.memset(trimask[:], 1.0)
            nc.gpsimd.affine_select(
                out=trimask[:], in_=trimask[:],
                compare_op=mybir.AluOpType.is_ge,
                fill=0.0, base=0,
                pattern=[[1, 128]], channel_multiplier=-1,
            )

            # ---- projections -----------------------------------------
            qT = [qkpool.tile([128, S], CDT, name=f"qT{m}") for m in range(MT)]
            kTt = [qkpool.tile([128, S], CDT, name=f"kT{m}") for m in range(MT)]
            ctxT = [qkpool.tile([128, S], CDT, name=f"ctxT{m}") for m in range(MT)]

            xqt = [xqa[:, k * S:(k + 1) * S] for k in range(8)]
            xkt = [xka[:, k * S:(k + 1) * S] for k in range(8)]
            xvt = [xva[:, k * S:(k + 1) * S] for k in range(8)]
            vaug = [vpool.tile([128, HG * 65], CDT, name=f"vaug{st}")
                    for st in range(KT)]

            def proj_qk_group(xt, wt, bt, dst, m, c):
                pt = pdpool.tile([128, 512], FP32, tag="pd", name="pt")
                for k in range(8):
                    nc.tensor.matmul(
                        pt[:],
                        wt[k][:, m * 128:(m + 1) * 128],
                        xt[k][:, c * 512:(c + 1) * 512],
                        start=(k == 0), stop=(k == 7),
                    )
                # eviction with per-partition bias on DVE
                nc.vector.tensor_scalar_add(
                    dst[m][:, c * 512:(c + 1) * 512], pt[:], bt[:, m:m + 1])

            def proj_v_group(st):
                # V in natural [seq, head-dim] layout, a ones column
                # appended per head for the softmax denominator.
                pv = pdpool.tile([128, 512], FP32, tag="pd", name="pv")
                for k in range(8):
                    nc.tensor.matmul(
                        pv[:, :DG],
                        xvt[k][:, st * 128:(st + 1) * 128],
                        wvt[k][:],
                        start=(k == 0), stop=(k == 7),
                    )
                va = vaug[st].rearrange("p (h c) -> p h c", h=HG)
                nc.vector.tensor_copy(
                    va[:, :, 0:DH],
                    pv[:, :DG].rearrange("p (h c) -> p h c", h=HG),
                )
                nc.vector.memset(va[:, :, DH:DH + 1], 1.0)

            # Q fully; K and V only what attention qc=0 needs. The rest
            # is emitted inside the qc loop so it fills ACT-bound stalls.
            for c in range(QC):
                for m in range(MT):
                    proj_qk_group(xqt, wqt, bqt, qT, m, c)
            for c in range(QC):
                for m in range(MT):
                    proj_qk_group(xkt, wkt, bkt, kTt, m, c)

            # dep-free filler: bridges PE idle between the end of
            # K-projection and the xv DMA arrival (uses an attention
            # PSUM slot, idle during projections).
            wps2 = sppool.tile([128, 1024], FP32, tag="sp", name="wps2")
            NFILL = 24
            for i in range(NFILL):
                nc.tensor.matmul(wps2[:, 0:512], wsrc[:, 0:128], wsrc[:],
                                 start=(i == 0), stop=(i == NFILL - 1))
            nc.vector.tensor_copy(wsb[:], wps2[0:1, 0:64])

            for st in range(4):
                proj_v_group(st)

            def dense_block(qt):
                for n in range(2):
                    dps = pdpool.tile([128, 512], FP32, tag="pd", name="dps")
                    for km in range(MT):
                        nc.tensor.matmul(
                            dps[:],
                            ctxT[km][:, qt * 128:(qt + 1) * 128],
                            wdt[km][:, n * 512:(n + 1) * 512],
                            start=(km == 0), stop=(km == MT - 1),
                        )
                    ot = opool.tile([128, 512], CDT, tag="ot", name="ot")
                    if n == 0:
                        nc.scalar.copy(ot[:], dps[:])
                    else:
                        nc.vector.tensor_copy(ot[:], dps[:])
                    nc.sync.dma_start(
                        out[qt * 128:(qt + 1) * 128, n * 512:(n + 1) * 512],
                        ot[:],
                    )

            # ---- attention + dense, interleaved per q-chunk ----------
            for qc in range(QC):
                nkt = 4 * qc + 4       # causal k tiles for this q chunk
                lastkt = nkt - 1
                for h in range(HG):
                    m, po = h // 2, 64 * (h % 2)
                    cps = pcpool.tile([65, 512], FP32, tag="cps", name="cps")

                    def emit_s(kp):
                        sp = sppool.tile([128, 1024], FP32, tag="sp", name="sp")
                        for j in range(2):
                            kt = 2 * kp + j
                            r = kt - 4 * qc
                            off = 128 * r if r > 0 else 0
                            nc.tensor.matmul(
                                sp[:, j * 512 + off:(j + 1) * 512],
                                kTt[m][po:po + DH, kt * 128:(kt + 1) * 128],
                                qT[m][po:po + DH,
                                      qc * 512 + off:(qc + 1) * 512],
                            )
                        return sp

                    def emit_ec(kp, sp):
                        es = spool.tile([128, 1024], CDT, tag="es", name="es")
                        offs = [max(0, 128 * (2 * kp + j - 4 * qc))
                                for j in range(2)]
                        if offs == [0, 0]:
                            nc.scalar.activation(es[:], sp[:], Exp, scale=0.125)
                        else:
                            for j in range(2):
                                sl = slice(j * 512 + offs[j], (j + 1) * 512)
                                nc.scalar.activation(es[:, sl], sp[:, sl],
                                                     Exp, scale=0.125)
                        for j in range(2):
                            kt = 2 * kp + j
                            r = kt - 4 * qc
                            off = 128 * r if r > 0 else 0
                            if r >= 0:
                                dj = es[:, j * 512 + off:j * 512 + off + 128]
                                nc.vector.tensor_mul(dj, dj, trimask[:])
                            nc.tensor.matmul(
                                cps[:, off:512],
                                vaug[kt][:, 65 * h:65 * h + 65],
                                es[:, j * 512 + off:(j + 1) * 512],
                                start=(kt == 0), stop=(kt == lastkt),
                            )

                    # software pipeline: S-pairs run one stage ahead of
                    # the exp/C chain so PE never waits on ACT latency.
                    prev = emit_s(0)
                    for kp in range(1, nkt // 2):
                        sp = emit_s(kp)
                        emit_ec(kp - 1, prev)
                        prev = sp
                    emit_ec(nkt // 2 - 1, prev)

                    den = small.tile([1, 512], FP32, tag="den", name="den")
                    nc.vector.tensor_copy(den[:], cps[DH:DH + 1, :])
                    recip = small.tile([1, 512], FP32, tag="recip", name="recip")
                    nc.vector.reciprocal_approx_fast(recip[:], den[:])
                    recipb = small.tile([DH, 512], FP32, tag="recipb", name="recipb")
                    nc.gpsimd.partition_broadcast(recipb[:], recip[:])
                    cslice = ctxT[m][po:po + DH, qc * 512:(qc + 1) * 512]
                    nc.vector.tensor_mul(cslice, cps[0:DH, :], recipb[:])
                    # fine-grained interleave: one V-prefetch group and
                    # one deferred dense block per head keeps ACT fed
                    # with exp work instead of idling through long
                    # PE-only bursts between q-chunks.
                    if qc < QC - 1:
                        proj_v_group(4 * qc + 4 + h)
                    if qc > 0:
                        dense_block(4 * (qc - 1) + h)

            # dep-free filler: keeps the PE warm while the last head's
            # normalize chain drains before the final dense blocks.
            wps3 = sppool.tile([128, 1024], FP32, tag="sp", name="wps3")
            for i in range(14):
                nc.tensor.matmul(wps3[:, 0:512], wsrc[:, 0:128], wsrc[:],
                                 start=(i == 0), stop=(i == 13))
            nc.vector.tensor_copy(wsb[:], wps3[0:1, 0:64])
            for qt in range(12, 16):
                dense_block(qt)

            # warmup result DMA last, on the gpsimd queue, so it cannot
            # stall input DMAs on the sync queue.
            nc.gpsimd.dma_start(wout[:], wsb[:])

    nc.compile()
    return nc


def get_nc():
    if "nc" not in _CACHE:
        _CACHE["nc"] = _build_nc()
    return _CACHE["nc"]


def make_in_maps(q, k, v, Wq_w, Wq_b, Wk_w, Wk_b, Wv_w, Wv_b, dense_w):
    bf16 = ml_dtypes.bfloat16
    f32 = np.float32

    def cast_t(a):
        # [S, D] -> transposed [D, S] -> partition-major k-tile layout
        # [128, 8*S] with row p holding k-tiles 0..7's partition p.
        t = np.asarray(a, f32).T.reshape(8, 128, S).transpose(1, 0, 2)
        return np.ascontiguousarray(t.reshape(128, 8 * S)).astype(bf16)

    def cast_w(wmat, gsl):
        # W[gsl].T [D, DG] -> [128, 8*DG] partition-major k-tiles
        t = np.asarray(wmat, f32)[gsl, :].T.reshape(8, 128, DG)
        return np.ascontiguousarray(
            t.transpose(1, 0, 2).reshape(128, 8 * DG)).astype(bf16)

    xq = [cast_t(q[b]) for b in range(B)]
    xk = [cast_t(k[b]) for b in range(B)]
    xv = [cast_t(v[b]) for b in range(B)]

    in_maps = []
    for core in range(NCORES):
        b, g = core // 4, core % 4
        gsl = slice(g * DG, (g + 1) * DG)
        m = {
            "xqT": xq[b], "xkT": xk[b], "xvT": xv[b],
            "wq": cast_w(Wq_w, gsl),
            "wk": cast_w(Wk_w, gsl),
            "wv": cast_w(Wv_w, gsl),
            "bq": np.ascontiguousarray(np.asarray(Wq_b, f32)[gsl].reshape(MT, 128).T),
            "bk": np.ascontiguousarray(np.asarray(Wk_b, f32)[gsl].reshape(MT, 128).T),
            "wd": np.ascontiguousarray(np.asarray(dense_w, f32)[:, gsl].T).astype(bf16),
        }
        in_maps.append(m)
    return in_maps


def gather(results, dense_b, Wv_b, dense_w):
    # V-bias is linear through the dense layer: ctx bias contributes the
    # constant row Wv_b @ dense_w.T, added here instead of on device.
    const_row = (np.asarray(Wv_b, np.float32) @
                 np.asarray(dense_w, np.float32).T + np.asarray(dense_b, np.float32))
    out = np.empty((B, S, D), np.float32)
    for b in range(B):
        acc = results[b * 4]["out"].astype(np.float32).copy()
        for g in range(1, 4):
            acc += results[b * 4 + g]["out"]
        out[b] = acc + const_row
    return out


def kernel(q, k, v, mask, Wq_w, Wq_b, Wk_w, Wk_b, Wv_w, Wv_b, dense_w, dense_b,
           trace=False):
    from concourse.bass_utils import run_bass_kernel_spmd

    nc = get_nc()
    in_maps = make_in_maps(q, k, v, Wq_w, Wq_b, Wk_w, Wk_b, Wv_w, Wv_b, dense_w)
    res = run_bass_kernel_spmd(nc, in_maps, list(range(NCORES)), trace=trace)
    out = gather(res.results, dense_b, Wv_b, dense_w)
    if trace:
        return out, res
    return out
